# revision 11
# baseline (speedup 1.0000x reference)
"""Trainium2 Bass kernel for nn_GTLayer_84722524880938.

The reference uses .reshape (not transpose) for the attention head split,
which makes attention block-diagonal over 256-row blocks of the sequence:
output rows [256n, 256n+256) depend only on input rows [256n, 256n+256)
(plus the full-length relative-position bias, which is rank-4). The layer
therefore shards perfectly across 8 cores: core c takes 512 contiguous
rows (2 blocks) of batch c//4 and needs no collectives.

Per 256-row block (X = h[b, 256n:256n+256, :]):
  q = X@Wq; k = X@Wk; v = X@Wv            [256, 1024]
  Q = q.reshape(2048, 128); KT = k.reshape(128, 2048); V = v.reshape(2048, 128)
  S = Q@KT/sqrt(128) + (rh[b]@Wrq) @ (rh[b]@Wrk).reshape(4, 2048) / 2
  P = softmax(S, -1);  C = P@V            [2048, 128]
  h_sa = C.reshape(256, 1024) @ Wo
  h1 = LN(h_sa + X);  hf = relu(h1@W1 + b1)@W2 + b2;  out = LN(h1 + hf)

All matmuls run as float32r (full fp32 data, full-rate PE mode). Scores
are exponentiated without max-subtraction (|S| < ~14, far from fp32 exp
overflow). The softmax denominator comes from an extra ones-weight matmul
accumulated alongside P@V.
"""

import sys

sys.path.insert(0, "/opt/trn_rl_repo")

import math

import numpy as np

import concourse.bass as bass
import concourse.mybir as mybir
import concourse.tile as tile
from concourse.bass_utils import run_bass_kernel_spmd
from concourse.masks import make_identity

F32 = mybir.dt.float32
F32R = mybir.dt.float32r
BF16 = mybir.dt.bfloat16

D, FFN, NH, HD, RL = 1024, 4096, 8, 128, 4
B, L = 2, 2048
ROWS = 512  # rows per core
NBLK = 2  # 256-row attention blocks per core
EPS = 1e-5
EXP_SCALE = 1.0 / math.sqrt(HD)  # applied by ACT on scores
RK_SCALE = math.sqrt(HD) / 2.0  # folded into r_k so bias lands as bias/2

MAX_WAITS = 1  # this walrus build allows one semaphore wait per instruction

_cache = {}


def _fix_waits(nc):
    """Split >MAX_WAITS sync waits onto injected same-engine NoOps.

    Engines execute their stream in order, so hoisting excess waits onto
    NoOps placed immediately before the instruction preserves semantics.
    """
    ctr = 0
    for f in nc.m.functions:
        for blk in f.blocks:
            out = []
            changed = False
            for ins in blk.instructions:
                si = ins.sync_info
                waits = list(si.on_wait) if si is not None else []
                if len(waits) > MAX_WAITS:
                    changed = True
                    while len(waits) > MAX_WAITS:
                        chunk, waits = waits[:MAX_WAITS], waits[MAX_WAITS:]
                        ctr += 1
                        nop = mybir.InstNoOp(
                            name=f"waitfix-nop-{ctr}",
                            ins=[],
                            outs=[],
                            sync_info=mybir.SyncInfo(on_wait=chunk, on_update=[]),
                        )
                        nop.engine = ins.engine
                        out.append(nop)
                    ins.sync_info = mybir.SyncInfo(
                        on_wait=waits, on_update=list(si.on_update)
                    )
                out.append(ins)
            if changed:
                blk.instructions = out
    return nc


def _r(ap):
    return ap.bitcast(F32R)


def _fview(base, free_dims, extra_off=0):
    """Rebuild an AP keeping the partition dim, with custom free dims/offset."""
    return bass.AP(
        tensor=base.tensor,
        offset=base.offset + extra_off,
        ap=[list(base.ap[0])] + [list(d) for d in free_dims],
    )


def build_nc(debug=False, repeat=1, phases=None):
    nc = bass.Bass(target_bir_lowering=False)

    x_d = nc.dram_tensor("x", [ROWS, D], F32, kind="ExternalInput")
    rh_d = nc.dram_tensor("rh", [L, RL], F32, kind="ExternalInput")
    wq_d = nc.dram_tensor("Wq", [D, D], F32, kind="ExternalInput")
    wk_d = nc.dram_tensor("Wk", [D, D], F32, kind="ExternalInput")
    wv_d = nc.dram_tensor("Wv", [D, D], F32, kind="ExternalInput")
    wo_d = nc.dram_tensor("Wo", [D, D], F32, kind="ExternalInput")
    wrk_d = nc.dram_tensor("Wrk", [RL, RL], F32, kind="ExternalInput")
    wrq_d = nc.dram_tensor("Wrq", [RL, RL], F32, kind="ExternalInput")
    w1_d = nc.dram_tensor("W1", [D, FFN], BF16, kind="ExternalInput")
    b1_d = nc.dram_tensor("b1", [FFN], F32, kind="ExternalInput")
    w2_d = nc.dram_tensor("W2", [FFN, D], BF16, kind="ExternalInput")
    b2_d = nc.dram_tensor("b2", [D], F32, kind="ExternalInput")
    g1_d = nc.dram_tensor("g1", [D], F32, kind="ExternalInput")
    be1_d = nc.dram_tensor("be1", [D], F32, kind="ExternalInput")
    g2_d = nc.dram_tensor("g2", [D], F32, kind="ExternalInput")
    be2_d = nc.dram_tensor("be2", [D], F32, kind="ExternalInput")
    out_d = nc.dram_tensor("out", [ROWS, D], F32, kind="ExternalOutput")

    dbg = {}
    if debug:
        dbg["qT"] = nc.dram_tensor("dbg_qT", [128, NH * ROWS], F32, kind="ExternalOutput")
        dbg["KT"] = nc.dram_tensor("dbg_KT", [128, 16, 128], F32, kind="ExternalOutput")
        dbg["V"] = nc.dram_tensor("dbg_V", [128, 16, 128], F32, kind="ExternalOutput")
        dbg["rkR"] = nc.dram_tensor("dbg_rkR", [RL, L], F32, kind="ExternalOutput")
        dbg["rqT"] = nc.dram_tensor("dbg_rqT", [RL, L], F32, kind="ExternalOutput")
        dbg["E"] = nc.dram_tensor("dbg_E", [128, 1024], F32, kind="ExternalOutput")
        dbg["CT"] = nc.dram_tensor("dbg_CT", [128, L], F32, kind="ExternalOutput")
        dbg["h1"] = nc.dram_tensor("dbg_h1", [128, 4, D], F32, kind="ExternalOutput")
        dbg["relu"] = nc.dram_tensor("dbg_relu", [128, ROWS], F32, kind="ExternalOutput")

    ph = phases
    with tile.TileContext(nc, pool_alloc_mode="stack") as tc:
        for _rep in range(repeat):
            _body(nc, tc, locals())

    _fix_waits(nc)
    return nc


def _body(nc, tc, t):
    phases = t["ph"] or {"qkv", "ktv", "attn", "wo", "ffn1", "ffn2"}
    debug = t["debug"]
    dbg = t["dbg"]
    x_d, rh_d, out_d = t["x_d"], t["rh_d"], t["out_d"]

    import contextlib

    ctx = contextlib.ExitStack()
    with ctx:
        # ---- pools ordered by lifetime (longest-lived first) ------------
        singles = ctx.enter_context(tc.tile_pool(name="singles", bufs=1))
        h1T_es = ctx.enter_context(contextlib.ExitStack())
        ct_es = h1T_es.enter_context(contextlib.ExitStack())
        qkv_es = ct_es.enter_context(contextlib.ExitStack())
        kv_es = qkv_es.enter_context(contextlib.ExitStack())

        ident = singles.tile([128, 128], F32)
        make_identity(nc, ident)
        ones_f = singles.tile([128, 128], F32, name="ones_f")
        nc.vector.memset(ones_f, 1.0)
        ones_sb = singles.tile([128, 128], F32R, name="ones_sb")
        nc.vector.tensor_copy(out=ones_sb, in_=ones_f)
        eps_t = singles.tile([128, 1], F32)
        nc.vector.memset(eps_t, EPS)

        def bcast_load(pool, dram, name):
            tl = pool.tile([128, D], F32, name=name, tag=name)
            src = bass.AP(tensor=dram, offset=0, ap=[[0, 128], [1, D]])
            nc.sync.dma_start(out=tl, in_=src)
            return tl

        b1t = singles.tile([128, FFN // 128], F32)
        nc.sync.dma_start(
            out=b1t,
            in_=bass.AP(tensor=t["b1_d"], offset=0, ap=[[1, 128], [128, FFN // 128]]),
        )
        h1_s = singles.tile([128, 4, D], F32, name="h1_s")  # written after LN1

        h1T_pool = h1T_es.enter_context(tc.tile_pool(name="h1T", bufs=1))
        h1T_s = h1T_pool.tile([128, 8, ROWS], BF16, name="h1T_s")

        ct_pool = ct_es.enter_context(tc.tile_pool(name="ct", bufs=1))
        CT_s = [ct_pool.tile([128, L], F32R, name=f"CTb{b}", tag=f"CTb{b}") for b in range(NBLK)]

        qT_pool = qkv_es.enter_context(tc.tile_pool(name="qT", bufs=1))
        qT_s = qT_pool.tile([128, NH * ROWS], F32R, name="qT_s")
        rqT_s = qT_pool.tile([RL, L], F32R, name="rqT_s")
        rkR_s = qT_pool.tile([RL, L], F32R, name="rkR_s")
        ktv_pool = qkv_es.enter_context(tc.tile_pool(name="ktv", bufs=1))
        KT_sb = [ktv_pool.tile([128, 16, 128], F32R, name=f"KTb{b}", tag=f"KTb{b}") for b in range(NBLK)]
        V_sb = [ktv_pool.tile([128, 16, 128], F32R, name=f"Vb{b}", tag=f"Vb{b}") for b in range(NBLK)]

        kv_pool = kv_es.enter_context(tc.tile_pool(name="kv", bufs=1))
        vT_s = kv_pool.tile([128, NH * ROWS], F32, name="vT_s")
        kstream = kv_es.enter_context(tc.tile_pool(name="kstream", bufs=3))

        # ---- phase 1+2: XT, rel-bias, q/k/v; then KT/V tiles ------------
        with (
            tc.tile_pool(name="xt", bufs=1) as xt_pool,
            tc.tile_pool(name="psT", bufs=2, space="PSUM") as psT,
            tc.tile_pool(name="psMM", bufs=4, space="PSUM") as psMM,
            tc.tile_pool(name="psT2", bufs=2, space="PSUM") as psT2,
            tc.tile_pool(name="wtile", bufs=9) as wpool,
            tc.tile_pool(name="cpy", bufs=3) as cpy,
        ):
            xT_s = xt_pool.tile([128, 8, ROWS], F32R, name="xT_s")
            for rc in range(4):
                xrow = cpy.tile([128, D], F32, tag="xrow", name="xrow")
                nc.sync.dma_start(out=xrow, in_=x_d[rc * 128 : (rc + 1) * 128, :])
                for ct_i in range(8):
                    p = psT.tile([128, 128], F32, tag="pst", name="pT")
                    nc.tensor.transpose(p, xrow[:, ct_i * 128 : (ct_i + 1) * 128], ident)
                    nc.vector.tensor_copy(
                        out=xT_s[:, ct_i, rc * 128 : (rc + 1) * 128], in_=p
                    )

            # rhT [4, 2048] via 16 PE transposes of [128, 4] row tiles
            rh_sb = cpy.tile([128, 16, RL], F32, tag="rh", name="rh_sb")
            nc.sync.dma_start(
                out=rh_sb, in_=rh_d[:, :].rearrange("(a p) u -> p a u", p=128)
            )
            rhT_s = xt_pool.tile([RL, L], F32R, name="rhT_s")
            for a in range(16):
                p = psT.tile([128, 128], F32, tag="pst", name="pT2")
                nc.tensor.transpose(p[:RL, :], rh_sb[:, a, :], ident)
                nc.vector.tensor_copy(
                    out=rhT_s[:, a * 128 : (a + 1) * 128], in_=p[:RL, :]
                )

            # r_qT / r_kT: [4, 2048] = Wr.T @ rh.T
            wr_sb = cpy.tile([RL, 2, RL], F32R, tag="wr", name="wr_sb")
            nc.sync.dma_start(out=wr_sb[:, 0, :], in_=t["wrq_d"][:, :].bitcast(F32R))
            nc.sync.dma_start(out=wr_sb[:, 1, :], in_=t["wrk_d"][:, :].bitcast(F32R))
            rkT_s = xt_pool.tile([RL, L], F32R, name="rkT_s")
            for half in range(4):
                sl = slice(half * 512, (half + 1) * 512)
                pq = psMM.tile([128, 512], F32, tag="qkv", name="pq")[:RL, :]
                nc.tensor.matmul(
                    pq, _r(wr_sb[:, 0, :]), _r(rhT_s[:, sl]), start=True, stop=True
                )
                nc.vector.tensor_copy(out=rqT_s[:, sl], in_=pq)
                pk = psMM.tile([128, 512], F32, tag="qkv", name="pk")[:RL, :]
                nc.tensor.matmul(
                    pk, _r(wr_sb[:, 1, :]), _r(rhT_s[:, sl]), start=True, stop=True
                )
                nc.vector.tensor_scalar_mul(out=rkT_s[:, sl], in0=pk, scalar1=RK_SCALE)

            # rkR[t, 4i+u] = rkT[u, 512t+i]  (reshape(4, 2048) of r_k)
            for tt in range(4):
                for u in range(RL):
                    nc.sync.dma_start(
                        out=_fview(rkR_s[tt : tt + 1, :], [[RL, 512]], u),
                        in_=rkT_s[u : u + 1, tt * 512 : (tt + 1) * 512],
                    )

            if debug:
                nc.sync.dma_start(out=dbg["rkR"][:, :], in_=rkR_s.bitcast(F32))
                nc.sync.dma_start(out=dbg["rqT"][:, :], in_=rqT_s.bitcast(F32))

            # q and v full [128, 8, ROWS]; weights loaded as [128, 512] half-rows
            for w_d, dest in ((t["wq_d"], qT_s), (t["wv_d"], vT_s)) if "qkv" in phases else ():
                for half in range(2):
                    wrows = [None] * 8
                    for ci in range(8):
                        wrow = wpool.tile([128, 512], F32R, tag="w", name="wrow")
                        nc.sync.dma_start(
                            out=wrow,
                            in_=w_d[
                                ci * 128 : (ci + 1) * 128,
                                half * 512 : (half + 1) * 512,
                            ].bitcast(F32R),
                        )
                        wrows[ci] = wrow
                    for col in range(4):
                        co = half * 4 + col
                        pm = psMM.tile([128, ROWS], F32, tag="qkv", name="pm")
                        for ci in range(8):
                            nc.tensor.matmul(
                                pm,
                                _r(wrows[ci][:, col * 128 : (col + 1) * 128]),
                                _r(xT_s[:, ci, :]),
                                start=(ci == 0),
                                stop=(ci == 7),
                            )
                        nc.vector.tensor_copy(
                            out=_fview(dest[:, :], [[8, ROWS]], co), in_=pm
                        )

            # k per-co streaming: each co slice feeds only KT tiles t%8==co
            for half in range(2 if "qkv" in phases else 0):
                wrows = [None] * 8
                for ci in range(8):
                    wrow = wpool.tile([128, 512], F32R, tag="w", name="wkrow")
                    nc.sync.dma_start(
                        out=wrow,
                        in_=t["wk_d"][
                            ci * 128 : (ci + 1) * 128, half * 512 : (half + 1) * 512
                        ].bitcast(F32R),
                    )
                    wrows[ci] = wrow
                for col in range(4):
                    co = half * 4 + col
                    pm = psMM.tile([128, ROWS], F32, tag="qkv", name="pmk")
                    for ci in range(8):
                        nc.tensor.matmul(
                            pm,
                            _r(wrows[ci][:, col * 128 : (col + 1) * 128]),
                            _r(xT_s[:, ci, :]),
                            start=(ci == 0),
                            stop=(ci == 7),
                        )
                    kco = kstream.tile([128, ROWS], F32, tag="kco", name="kco")
                    nc.vector.tensor_copy(out=kco, in_=pm)
                    for b in range(NBLK):
                        for tt in (co, co + 8):
                            # KT_t^T[mm, d] = k[256b + 2d + (t>=8), 128co + mm]
                            kt_view = _fview(
                                kco[:, :], [[2, 128]], 256 * b + (1 if tt >= 8 else 0)
                            )
                            p = psT2.tile([128, 128], F32, tag="pst2", name="pKT")
                            nc.tensor.transpose(p, kt_view, ident)
                            nc.vector.tensor_copy(out=KT_sb[b][:, tt, :], in_=p)

            # V tiles from vT_s
            for b in range(NBLK if "ktv" in phases else 0):
                for tt in range(16):
                    # V_t^T[e, 8a+j] = vT_s[e, j, 256b + 16t + a]
                    v_view = _fview(
                        vT_s[:, :], [[1, 128]], 8 * (256 * b + 16 * tt)
                    )
                    pv = psT2.tile([128, 128], F32, tag="pst2", name="pV")
                    nc.tensor.transpose(pv, v_view, ident)
                    nc.vector.tensor_copy(out=V_sb[b][:, tt, :], in_=pv)
            if debug:
                nc.sync.dma_start(out=dbg["qT"][:, :], in_=qT_s.bitcast(F32))
                nc.sync.dma_start(out=dbg["KT"][:, :, :], in_=KT_sb[0].bitcast(F32))
                nc.sync.dma_start(out=dbg["V"][:, :, :], in_=V_sb[0].bitcast(F32))
        kv_es.close()  # vT/k-stream dead once KT/V tiles exist

        # Wo preload: region reuses the kv pool space (freed at P3 end), so
        # this 4MB DMA overlaps the whole attention phase. Lives in qkv_es,
        # whose close moves to after the Wo phase to keep LIFO order.
        wopool = qkv_es.enter_context(tc.tile_pool(name="wotile", bufs=1))
        wo_s = wopool.tile([128, 8, D], F32R, name="wo_s")
        nc.sync.dma_start(
            out=wo_s,
            in_=t["wo_d"][:, :].rearrange("(j p) n -> p j n", p=128).bitcast(F32R),
        )
        g1b = bcast_load(wopool, t["g1_d"], "g1b")
        be1b = bcast_load(wopool, t["be1_d"], "be1b")

        def layer_norm(dest, pre, gb, bb, pool):
            """dest = LN(pre) * gb + bb ; pre is [128, 1024] SBUF."""
            st = pool.tile([128, 2, 6], F32, tag="bnst", name="st")
            nc.vector.bn_stats(out=st[:, 0, :], in_=pre[:, 0:512])
            nc.vector.bn_stats(out=st[:, 1, :], in_=pre[:, 512:1024])
            mv = pool.tile([128, 2], F32, tag="bnmv", name="mv")
            nc.vector.bn_aggr(out=mv, in_=st)
            rstd = pool.tile([128, 1], F32, tag="rstd", name="rstd")
            nc.scalar.activation(
                out=rstd,
                in_=mv[:, 1:2],
                func=mybir.ActivationFunctionType.Sqrt,
                bias=eps_t,
            )
            nc.vector.reciprocal(out=rstd, in_=rstd)
            xn = pool.tile([128, D], F32, tag="xn", name="xn")
            nc.vector.tensor_scalar(
                out=xn,
                in0=pre,
                scalar1=mv[:, 0:1],
                scalar2=rstd,
                op0=mybir.AluOpType.subtract,
                op1=mybir.AluOpType.mult,
            )
            tmp = pool.tile([128, D], F32, tag="pre", name="tmp")
            nc.vector.tensor_mul(out=tmp, in0=xn, in1=gb)
            nc.vector.tensor_add(out=dest, in0=tmp, in1=bb)

        # ---- phases 4+5 fused: attention, then per-block Wo + LN1 -------
        # Wo shares the attention pool scope so block 0's Wo matmuls overlap
        # block 1's attention (PSUM: S 2 + C 2 + D 2 + Wo 2 = 8 banks).
        with (
            tc.tile_pool(name="psS", bufs=2, space="PSUM") as psS,
            tc.tile_pool(name="psC", bufs=1, space="PSUM") as psC,
            tc.tile_pool(name="psD", bufs=1, space="PSUM") as psD,
            tc.tile_pool(name="psWo", bufs=2, space="PSUM") as psWo,
            tc.tile_pool(name="epool", bufs=3) as epool,
            tc.tile_pool(name="inv", bufs=2) as invp,
            tc.tile_pool(name="lnp", bufs=2) as lnp,
        ):
            for b in range(NBLK if "attn" in phases else 0):
                for lh in range(2):
                    pC = psC.tile([128, 1024], F32, tag="pc", name="pC")
                    pD = psD.tile([128, 1024], F32, tag="pd", name="pD")
                    for tt in range(16):
                        e_t = epool.tile([128, 1024], F32R, tag="e", name="e_t")
                        for q in range(2):
                            lq = slice(q * 512, (q + 1) * 512)
                            pS = psS.tile([128, 512], F32, tag="ps", name="pS")
                            # l = 1024*lh + 512*q + 8r + j ; r0 = 128*lh + 64*q
                            off = 8 * (256 * b + 128 * lh + 64 * q)
                            qt_view = qT_s[:, off : off + 512]
                            nc.tensor.matmul(
                                pS,
                                _r(KT_sb[b][:, tt, :]),
                                _r(qt_view),
                                start=True,
                                stop=False,
                            )
                            nc.tensor.matmul(
                                pS,
                                _r(rkR_s[:, tt * 128 : (tt + 1) * 128]),
                                _r(rqT_s[:, 1024 * lh + 512 * q :][:, :512]),
                                start=False,
                                stop=True,
                            )
                            nc.scalar.activation(
                                out=e_t[:, lq],
                                in_=pS,
                                func=mybir.ActivationFunctionType.Exp,
                                scale=EXP_SCALE,
                            )
                        if debug and b == 0 and lh == 0 and tt == 0:
                            nc.sync.dma_start(out=dbg["E"][:, :], in_=e_t.bitcast(F32))
                        for q in range(2):
                            lq = slice(q * 512, (q + 1) * 512)
                            nc.tensor.matmul(
                                pC[:, lq],
                                _r(V_sb[b][:, tt, :]),
                                _r(e_t[:, lq]),
                                start=(tt == 0),
                                stop=(tt == 15),
                            )
                            nc.tensor.matmul(
                                pD[:, lq],
                                _r(ones_sb),
                                _r(e_t[:, lq]),
                                start=(tt == 0),
                                stop=(tt == 15),
                            )
                    inv_t = invp.tile([128, 1024], F32, tag="inv", name="inv_t")
                    nc.vector.reciprocal(out=inv_t, in_=pD)
                    nc.vector.tensor_mul(
                        out=CT_s[b][:, 1024 * lh : 1024 * (lh + 1)],
                        in0=pC,
                        in1=inv_t,
                    )
                    # Wo + residual + LN1 for this 128-row chunk: its Wo
                    # matmuls read only the lh half of CT, so they overlap
                    # the other lh / next block instead of serializing after.
                    rc2 = lh
                    if "wo" not in phases:
                        continue
                    a = 2 * b + rc2  # core row-chunk index
                    xrow = lnp.tile([128, D], F32, tag="xrow2", name="xrow2")
                    nc.sync.dma_start(
                        out=xrow, in_=x_d[a * 128 : (a + 1) * 128, :]
                    )
                    pre = lnp.tile([128, D], F32, tag="pre", name="pre")
                    for nchunk in range(2):
                        ph = psWo.tile([128, 512], F32, tag="pswo", name="ph")
                        for j in range(8):
                            ctx_view = _fview(
                                CT_s[b][:, :], [[8, 128]], 1024 * rc2 + j
                            )
                            nc.tensor.matmul(
                                ph,
                                _r(ctx_view),
                                _r(wo_s[:, j, nchunk * 512 : (nchunk + 1) * 512]),
                                start=(j == 0),
                                stop=(j == 7),
                            )
                        nc.vector.tensor_add(
                            out=pre[:, nchunk * 512 : (nchunk + 1) * 512],
                            in0=ph,
                            in1=xrow[:, nchunk * 512 : (nchunk + 1) * 512],
                        )
                    layer_norm(h1_s[:, a, :], pre, g1b, be1b, lnp)
            if debug:
                nc.sync.dma_start(out=dbg["CT"][:, :], in_=CT_s[0].bitcast(F32))
                nc.sync.dma_start(out=dbg["h1"][:, :, :], in_=h1_s)
        qkv_es.close()  # qT/rel/KT/V/Wo dead after Wo+LN1
        ct_es.close()  # CT dead after Wo

        # ---- FFN prelude: W2/bias prefetch + h1+b2 precompute -----------
        # Opens right after the attention pools close, so the 8MB of bf16 W2
        # streams during the h1-transpose + FFN1 phases.
        ffn_es = ctx.enter_context(contextlib.ExitStack())
        w2keep = ffn_es.enter_context(tc.tile_pool(name="w2keep", bufs=1))
        rkeep = ffn_es.enter_context(tc.tile_pool(name="rkeep", bufs=32))
        lnconst = ffn_es.enter_context(tc.tile_pool(name="lnconst", bufs=1))
        ln2p = ffn_es.enter_context(tc.tile_pool(name="ln2p", bufs=2))
        outp = ffn_es.enter_context(tc.tile_pool(name="outp", bufs=2))
        pacca_pool = ffn_es.enter_context(
            tc.tile_pool(name="psFa", bufs=1, space="PSUM")
        )

        w2_tiles = []
        for fg in range(8):
            w2g = w2keep.tile([128, 4, D], BF16, tag=f"w2_{fg}", name=f"w2g{fg}")
            nc.sync.dma_start(
                out=w2g,
                in_=t["w2_d"][fg * 512 : (fg + 1) * 512, :].rearrange(
                    "(g p) c -> p g c", p=128
                ),
            )
            w2_tiles.append(w2g)
        g2b = bcast_load(lnconst, t["g2_d"], "g2b")
        be2b = bcast_load(lnconst, t["be2_d"], "be2b")
        b2b = bcast_load(lnconst, t["b2_d"], "b2b")
        h1b2_s = lnconst.tile([128, 4, D], F32, name="h1b2_s")
        for a in range(4):
            nc.vector.tensor_add(out=h1b2_s[:, a, :], in0=h1_s[:, a, :], in1=b2b)

        # ---- phase 6: h1T (bf16) ----------------------------------------
        with tc.tile_pool(name="psT3", bufs=2, space="PSUM") as psT3:
            for ct_i in range(8):
                for a in range(4):
                    p = psT3.tile([128, 128], F32, tag="pst3", name="pH")
                    nc.tensor.transpose(
                        p, h1_s[:, a, ct_i * 128 : (ct_i + 1) * 128], ident
                    )
                    nc.vector.tensor_copy(
                        out=h1T_s[:, ct_i, a * 128 : (a + 1) * 128], in_=p
                    )

        # ---- phase 7: FFN1+relu with FFN2 rows 0-255 interleaved --------
        pacc_a = [
            pacca_pool.tile([128, 512], F32, tag=f"psfa_{i}", name=f"psfa_{i}")
            for i in range(4)
        ]
        relu_tiles = [None] * 32

        def ffn2_rows(f, pacc_pair01, a_base):
            fg, fl = f // 4, f % 4
            rl_t = relu_tiles[f]
            for cchunk in range(2):
                for ai in range(2):
                    nc.tensor.matmul(
                        pacc_pair01[ai * 2 + cchunk],
                        rl_t[:, (a_base + ai) * 128 : (a_base + ai + 1) * 128],
                        w2_tiles[fg][:, fl, cchunk * 512 : (cchunk + 1) * 512],
                        start=(f == 0),
                        stop=(f == 31),
                    )

        def ln2_out(a, pacc_pair):
            pre2 = ln2p.tile([128, D], F32, tag="pre", name="pre2")
            for cchunk in range(2):
                cs = slice(cchunk * 512, (cchunk + 1) * 512)
                nc.vector.tensor_add(
                    out=pre2[:, cs],
                    in0=pacc_pair[cchunk],
                    in1=h1b2_s[:, a, cs],
                )
            o_t = outp.tile([128, D], F32, tag="o", name="o_t")
            layer_norm(o_t, pre2, g2b, be2b, ln2p)
            nc.sync.dma_start(out=out_d[a * 128 : (a + 1) * 128, :], in_=o_t)

        with (
            tc.tile_pool(name="psF1", bufs=4, space="PSUM") as psF1,
            tc.tile_pool(name="w1tile", bufs=12) as w1pool,
        ):
            w1rows = [None] * 8
            for f in range(32 if "ffn1" in phases else 0):
                fq, fl = f // 8, f % 8
                if fl == 0:
                    for ci in range(8):
                        wt = w1pool.tile([128, 1024], BF16, tag="w1", name="w1t")
                        nc.sync.dma_start(
                            out=wt,
                            in_=t["w1_d"][
                                ci * 128 : (ci + 1) * 128,
                                fq * 1024 : (fq + 1) * 1024,
                            ],
                        )
                        w1rows[ci] = wt
                pm = psF1.tile([128, ROWS], F32, tag="psf1", name="pF")
                for ci in range(8):
                    nc.tensor.matmul(
                        pm,
                        w1rows[ci][:, fl * 128 : (fl + 1) * 128],
                        h1T_s[:, ci, :],
                        start=(ci == 0),
                        stop=(ci == 7),
                    )
                rt = rkeep.tile([128, ROWS], BF16, tag="rkeep", name="rk")
                relu_tiles[f] = rt
                nc.scalar.activation(
                    out=rt,
                    in_=pm,
                    func=mybir.ActivationFunctionType.Relu,
                    bias=b1t[:, f : f + 1],
                )
                if f > 0 and "ffn2" in phases:
                    ffn2_rows(f - 1, pacc_a, 0)
            if "ffn2" in phases:
                ffn2_rows(31, pacc_a, 0)

        # ---- phase 8: FFN2 rows 256-511 (PE) overlapped with LN2 a=0,1 --
        with tc.tile_pool(name="psFb", bufs=1, space="PSUM") as paccb_pool:
            pacc_b = [
                paccb_pool.tile([128, 512], F32, tag=f"psfb_{i}", name=f"psfb_{i}")
                for i in range(4)
            ]
            for a in range(2 if "ffn2" in phases else 0):
                ln2_out(a, pacc_a[2 * a : 2 * a + 2])
            for f in range(32 if "ffn2" in phases else 0):
                ffn2_rows(f, pacc_b, 2)
            for a in range(2, 4) if "ffn2" in phases else ():
                ln2_out(a, pacc_b[2 * (a - 2) : 2 * (a - 2) + 2])


def _get_nc(debug=False):
    key = ("dbg" if debug else "main")
    if key not in _cache:
        _cache[key] = build_nc(debug)
    return _cache[key]


def kernel(**inputs):
    import ml_dtypes

    h = np.ascontiguousarray(np.asarray(inputs["h"], dtype=np.float32))
    rh = np.ascontiguousarray(np.asarray(inputs["rh"], dtype=np.float32))
    weights = {
        k: np.ascontiguousarray(np.asarray(inputs[k], dtype=np.float32))
        for k in (
            "Wq", "Wk", "Wv", "Wo", "Wrk", "Wrq",
            "b1", "b2", "g1", "be1", "g2", "be2",
        )
    }
    for k in ("W1", "W2"):
        weights[k] = np.ascontiguousarray(
            np.asarray(inputs[k]).astype(ml_dtypes.bfloat16)
        )
    in_maps = []
    for c in range(8):
        b, r0 = c // 4, 512 * (c % 4)
        m = {"x": h[b, r0 : r0 + 512, :], "rh": rh[b]}
        m.update(weights)
        in_maps.append(m)

    nc = _get_nc()
    res = run_bass_kernel_spmd(nc, in_maps, core_ids=list(range(8)))
    out = np.empty((B, L, D), dtype=np.float32)
    for c in range(8):
        b, r0 = c // 4, 512 * (c % 4)
        out[b, r0 : r0 + 512, :] = res.results[c]["out"]
    return out



# revision 13
# speedup vs baseline: 1.0300x; 1.0300x over previous
"""Trainium2 Bass kernel for nn_GTLayer_84722524880938.

The reference uses .reshape (not transpose) for the attention head split,
which makes attention block-diagonal over 256-row blocks of the sequence:
output rows [256n, 256n+256) depend only on input rows [256n, 256n+256)
(plus the full-length relative-position bias, which is rank-4). The layer
therefore shards perfectly across 8 cores: core c takes 512 contiguous
rows (2 blocks) of batch c//4 and needs no collectives.

Per 256-row block (X = h[b, 256n:256n+256, :]):
  q = X@Wq; k = X@Wk; v = X@Wv            [256, 1024]
  Q = q.reshape(2048, 128); KT = k.reshape(128, 2048); V = v.reshape(2048, 128)
  S = Q@KT/sqrt(128) + (rh[b]@Wrq) @ (rh[b]@Wrk).reshape(4, 2048) / 2
  P = softmax(S, -1);  C = P@V            [2048, 128]
  h_sa = C.reshape(256, 1024) @ Wo
  h1 = LN(h_sa + X);  hf = relu(h1@W1 + b1)@W2 + b2;  out = LN(h1 + hf)

All matmuls run as float32r (full fp32 data, full-rate PE mode). Scores
are exponentiated without max-subtraction (|S| < ~14, far from fp32 exp
overflow). The softmax denominator comes from an extra ones-weight matmul
accumulated alongside P@V.
"""

import sys

sys.path.insert(0, "/opt/trn_rl_repo")

import math

import numpy as np

import concourse.bass as bass
import concourse.mybir as mybir
import concourse.tile as tile
from concourse.bass_utils import run_bass_kernel_spmd
from concourse.masks import make_identity

F32 = mybir.dt.float32
F32R = mybir.dt.float32r
BF16 = mybir.dt.bfloat16

D, FFN, NH, HD, RL = 1024, 4096, 8, 128, 4
B, L = 2, 2048
ROWS = 512  # rows per core
NBLK = 2  # 256-row attention blocks per core
EPS = 1e-5
EXP_SCALE = 1.0 / math.sqrt(HD)  # applied by ACT on scores
RK_SCALE = math.sqrt(HD) / 2.0  # folded into r_k so bias lands as bias/2

MAX_WAITS = 1  # this walrus build allows one semaphore wait per instruction

_cache = {}


def _fix_waits(nc):
    """Split >MAX_WAITS sync waits onto injected same-engine NoOps.

    Engines execute their stream in order, so hoisting excess waits onto
    NoOps placed immediately before the instruction preserves semantics.
    """
    ctr = 0
    for f in nc.m.functions:
        for blk in f.blocks:
            out = []
            changed = False
            for ins in blk.instructions:
                si = ins.sync_info
                waits = list(si.on_wait) if si is not None else []
                if len(waits) > MAX_WAITS:
                    changed = True
                    while len(waits) > MAX_WAITS:
                        chunk, waits = waits[:MAX_WAITS], waits[MAX_WAITS:]
                        ctr += 1
                        nop = mybir.InstNoOp(
                            name=f"waitfix-nop-{ctr}",
                            ins=[],
                            outs=[],
                            sync_info=mybir.SyncInfo(on_wait=chunk, on_update=[]),
                        )
                        nop.engine = ins.engine
                        out.append(nop)
                    ins.sync_info = mybir.SyncInfo(
                        on_wait=waits, on_update=list(si.on_update)
                    )
                out.append(ins)
            if changed:
                blk.instructions = out
    return nc


def _r(ap):
    return ap.bitcast(F32R)


def _fview(base, free_dims, extra_off=0):
    """Rebuild an AP keeping the partition dim, with custom free dims/offset."""
    return bass.AP(
        tensor=base.tensor,
        offset=base.offset + extra_off,
        ap=[list(base.ap[0])] + [list(d) for d in free_dims],
    )


def build_nc(debug=False, repeat=1, phases=None):
    nc = bass.Bass(target_bir_lowering=False)

    x_d = nc.dram_tensor("x", [ROWS, D], F32, kind="ExternalInput")
    rh_d = nc.dram_tensor("rh", [L, RL], F32, kind="ExternalInput")
    wq_d = nc.dram_tensor("Wq", [D, D], F32, kind="ExternalInput")
    wk_d = nc.dram_tensor("Wk", [D, D], F32, kind="ExternalInput")
    wv_d = nc.dram_tensor("Wv", [D, D], F32, kind="ExternalInput")
    wo_d = nc.dram_tensor("Wo", [D, D], F32, kind="ExternalInput")
    wrk_d = nc.dram_tensor("Wrk", [RL, RL], F32, kind="ExternalInput")
    wrq_d = nc.dram_tensor("Wrq", [RL, RL], F32, kind="ExternalInput")
    w1_d = nc.dram_tensor("W1", [D, FFN], BF16, kind="ExternalInput")
    b1_d = nc.dram_tensor("b1", [FFN], F32, kind="ExternalInput")
    w2_d = nc.dram_tensor("W2", [FFN, D], BF16, kind="ExternalInput")
    b2_d = nc.dram_tensor("b2", [D], F32, kind="ExternalInput")
    g1_d = nc.dram_tensor("g1", [D], F32, kind="ExternalInput")
    be1_d = nc.dram_tensor("be1", [D], F32, kind="ExternalInput")
    g2_d = nc.dram_tensor("g2", [D], F32, kind="ExternalInput")
    be2_d = nc.dram_tensor("be2", [D], F32, kind="ExternalInput")
    out_d = nc.dram_tensor("out", [ROWS, D], F32, kind="ExternalOutput")

    dbg = {}
    if debug:
        dbg["qT"] = nc.dram_tensor("dbg_qT", [128, NH * ROWS], F32, kind="ExternalOutput")
        dbg["KT"] = nc.dram_tensor("dbg_KT", [128, 16, 128], F32, kind="ExternalOutput")
        dbg["V"] = nc.dram_tensor("dbg_V", [128, 16, 128], F32, kind="ExternalOutput")
        dbg["rkR"] = nc.dram_tensor("dbg_rkR", [RL, L], F32, kind="ExternalOutput")
        dbg["rqT"] = nc.dram_tensor("dbg_rqT", [RL, L], F32, kind="ExternalOutput")
        dbg["E"] = nc.dram_tensor("dbg_E", [128, 1024], F32, kind="ExternalOutput")
        dbg["CT"] = nc.dram_tensor("dbg_CT", [128, L], F32, kind="ExternalOutput")
        dbg["h1"] = nc.dram_tensor("dbg_h1", [128, 4, D], F32, kind="ExternalOutput")
        dbg["relu"] = nc.dram_tensor("dbg_relu", [128, ROWS], F32, kind="ExternalOutput")

    ph = phases
    with tile.TileContext(nc, pool_alloc_mode="stack") as tc:
        for _rep in range(repeat):
            _body(nc, tc, locals())

    _fix_waits(nc)
    return nc


def _body(nc, tc, t):
    phases = t["ph"] or {"qkv", "ktv", "attn", "wo", "ffn1", "ffn2"}
    debug = t["debug"]
    dbg = t["dbg"]
    x_d, rh_d, out_d = t["x_d"], t["rh_d"], t["out_d"]

    import contextlib

    ctx = contextlib.ExitStack()
    with ctx:
        # ---- pools ordered by lifetime (longest-lived first) ------------
        singles = ctx.enter_context(tc.tile_pool(name="singles", bufs=1))
        h1T_es = ctx.enter_context(contextlib.ExitStack())
        ct_es = h1T_es.enter_context(contextlib.ExitStack())
        qkv_es = ct_es.enter_context(contextlib.ExitStack())
        kv_es = qkv_es.enter_context(contextlib.ExitStack())

        ident = singles.tile([128, 128], F32)
        make_identity(nc, ident)
        ones_f = singles.tile([128, 128], F32, name="ones_f")
        nc.vector.memset(ones_f, 1.0)
        ones_sb = singles.tile([128, 128], F32R, name="ones_sb")
        nc.vector.tensor_copy(out=ones_sb, in_=ones_f)
        eps_t = singles.tile([128, 1], F32)
        nc.vector.memset(eps_t, EPS)

        def bcast_load(pool, dram, name):
            tl = pool.tile([128, D], F32, name=name, tag=name)
            src = bass.AP(tensor=dram, offset=0, ap=[[0, 128], [1, D]])
            nc.sync.dma_start(out=tl, in_=src)
            return tl

        b1t = singles.tile([128, FFN // 128], F32)
        nc.sync.dma_start(
            out=b1t,
            in_=bass.AP(tensor=t["b1_d"], offset=0, ap=[[1, 128], [128, FFN // 128]]),
        )
        h1_s = singles.tile([128, 4, D], F32, name="h1_s")  # written after LN1

        h1T_pool = h1T_es.enter_context(tc.tile_pool(name="h1T", bufs=1))
        h1T_s = h1T_pool.tile([128, 8, ROWS], BF16, name="h1T_s")

        ct_pool = ct_es.enter_context(tc.tile_pool(name="ct", bufs=1))
        CT_s = [ct_pool.tile([128, L], F32R, name=f"CTb{b}", tag=f"CTb{b}") for b in range(NBLK)]

        qT_pool = qkv_es.enter_context(tc.tile_pool(name="qT", bufs=1))
        qT_s = qT_pool.tile([128, NH * ROWS], F32R, name="qT_s")
        rqT_s = qT_pool.tile([RL, L], F32R, name="rqT_s")
        rkR_s = qT_pool.tile([RL, L], F32R, name="rkR_s")
        ktv_pool = qkv_es.enter_context(tc.tile_pool(name="ktv", bufs=1))
        KT_sb = [ktv_pool.tile([128, 16, 128], F32R, name=f"KTb{b}", tag=f"KTb{b}") for b in range(NBLK)]
        V_sb = [ktv_pool.tile([128, 16, 128], F32R, name=f"Vb{b}", tag=f"Vb{b}") for b in range(NBLK)]

        kv_pool = kv_es.enter_context(tc.tile_pool(name="kv", bufs=1))
        vT_s = kv_pool.tile([128, NH * ROWS], F32, name="vT_s")
        kstream = kv_es.enter_context(tc.tile_pool(name="kstream", bufs=3))

        # ---- phase 1+2: XT, rel-bias, q/k/v; then KT/V tiles ------------
        with (
            tc.tile_pool(name="xt", bufs=1) as xt_pool,
            tc.tile_pool(name="psT", bufs=2, space="PSUM") as psT,
            tc.tile_pool(name="psMM", bufs=4, space="PSUM") as psMM,
            tc.tile_pool(name="psT2", bufs=2, space="PSUM") as psT2,
            tc.tile_pool(name="wtile", bufs=9) as wpool,
            tc.tile_pool(name="cpy", bufs=3) as cpy,
        ):
            xT_s = xt_pool.tile([128, 8, ROWS], F32R, name="xT_s")
            for rc in range(4):
                xrow = cpy.tile([128, D], F32, tag="xrow", name="xrow")
                nc.sync.dma_start(out=xrow, in_=x_d[rc * 128 : (rc + 1) * 128, :])
                for ct_i in range(8):
                    p = psT.tile([128, 128], F32, tag="pst", name="pT")
                    nc.tensor.transpose(p, xrow[:, ct_i * 128 : (ct_i + 1) * 128], ident)
                    nc.vector.tensor_copy(
                        out=xT_s[:, ct_i, rc * 128 : (rc + 1) * 128], in_=p
                    )

            # rhT [4, 2048] via 16 PE transposes of [128, 4] row tiles
            rh_sb = cpy.tile([128, 16, RL], F32, tag="rh", name="rh_sb")
            nc.sync.dma_start(
                out=rh_sb, in_=rh_d[:, :].rearrange("(a p) u -> p a u", p=128)
            )
            rhT_s = xt_pool.tile([RL, L], F32R, name="rhT_s")
            for a in range(16):
                p = psT.tile([128, 128], F32, tag="pst", name="pT2")
                nc.tensor.transpose(p[:RL, :], rh_sb[:, a, :], ident)
                nc.vector.tensor_copy(
                    out=rhT_s[:, a * 128 : (a + 1) * 128], in_=p[:RL, :]
                )

            # r_qT / r_kT: [4, 2048] = Wr.T @ rh.T
            wr_sb = cpy.tile([RL, 2, RL], F32R, tag="wr", name="wr_sb")
            nc.sync.dma_start(out=wr_sb[:, 0, :], in_=t["wrq_d"][:, :].bitcast(F32R))
            nc.sync.dma_start(out=wr_sb[:, 1, :], in_=t["wrk_d"][:, :].bitcast(F32R))
            rkT_s = xt_pool.tile([RL, L], F32R, name="rkT_s")
            for half in range(4):
                sl = slice(half * 512, (half + 1) * 512)
                pq = psMM.tile([128, 512], F32, tag="qkv", name="pq")[:RL, :]
                nc.tensor.matmul(
                    pq, _r(wr_sb[:, 0, :]), _r(rhT_s[:, sl]), start=True, stop=True
                )
                nc.vector.tensor_copy(out=rqT_s[:, sl], in_=pq)
                pk = psMM.tile([128, 512], F32, tag="qkv", name="pk")[:RL, :]
                nc.tensor.matmul(
                    pk, _r(wr_sb[:, 1, :]), _r(rhT_s[:, sl]), start=True, stop=True
                )
                nc.vector.tensor_scalar_mul(out=rkT_s[:, sl], in0=pk, scalar1=RK_SCALE)

            # rkR[t, 4i+u] = rkT[u, 512t+i]  (reshape(4, 2048) of r_k)
            for tt in range(4):
                for u in range(RL):
                    nc.sync.dma_start(
                        out=_fview(rkR_s[tt : tt + 1, :], [[RL, 512]], u),
                        in_=rkT_s[u : u + 1, tt * 512 : (tt + 1) * 512],
                    )

            if debug:
                nc.sync.dma_start(out=dbg["rkR"][:, :], in_=rkR_s.bitcast(F32))
                nc.sync.dma_start(out=dbg["rqT"][:, :], in_=rqT_s.bitcast(F32))

            # q and v full [128, 8, ROWS]; weights loaded as [128, 512] half-rows
            for w_d, dest in ((t["wq_d"], qT_s), (t["wv_d"], vT_s)) if "qkv" in phases else ():
                for half in range(2):
                    wrows = [None] * 8
                    for ci in range(8):
                        wrow = wpool.tile([128, 512], F32R, tag="w", name="wrow")
                        nc.sync.dma_start(
                            out=wrow,
                            in_=w_d[
                                ci * 128 : (ci + 1) * 128,
                                half * 512 : (half + 1) * 512,
                            ].bitcast(F32R),
                        )
                        wrows[ci] = wrow
                    for col in range(4):
                        co = half * 4 + col
                        pm = psMM.tile([128, ROWS], F32, tag="qkv", name="pm")
                        for ci in range(8):
                            nc.tensor.matmul(
                                pm,
                                _r(wrows[ci][:, col * 128 : (col + 1) * 128]),
                                _r(xT_s[:, ci, :]),
                                start=(ci == 0),
                                stop=(ci == 7),
                            )
                        nc.vector.tensor_copy(
                            out=_fview(dest[:, :], [[8, ROWS]], co), in_=pm
                        )

            # k per-co streaming: each co slice feeds only KT tiles t%8==co
            for half in range(2 if "qkv" in phases else 0):
                wrows = [None] * 8
                for ci in range(8):
                    wrow = wpool.tile([128, 512], F32R, tag="w", name="wkrow")
                    nc.sync.dma_start(
                        out=wrow,
                        in_=t["wk_d"][
                            ci * 128 : (ci + 1) * 128, half * 512 : (half + 1) * 512
                        ].bitcast(F32R),
                    )
                    wrows[ci] = wrow
                for col in range(4):
                    co = half * 4 + col
                    pm = psMM.tile([128, ROWS], F32, tag="qkv", name="pmk")
                    for ci in range(8):
                        nc.tensor.matmul(
                            pm,
                            _r(wrows[ci][:, col * 128 : (col + 1) * 128]),
                            _r(xT_s[:, ci, :]),
                            start=(ci == 0),
                            stop=(ci == 7),
                        )
                    kco = kstream.tile([128, ROWS], F32, tag="kco", name="kco")
                    nc.vector.tensor_copy(out=kco, in_=pm)
                    for b in range(NBLK):
                        for tt in (co, co + 8):
                            # KT_t^T[mm, d] = k[256b + 2d + (t>=8), 128co + mm]
                            kt_view = _fview(
                                kco[:, :], [[2, 128]], 256 * b + (1 if tt >= 8 else 0)
                            )
                            p = psT2.tile([128, 128], F32, tag="pst2", name="pKT")
                            nc.tensor.transpose(p, kt_view, ident)
                            nc.vector.tensor_copy(out=KT_sb[b][:, tt, :], in_=p)

            # V tiles from vT_s
            for b in range(NBLK if "ktv" in phases else 0):
                for tt in range(16):
                    # V_t^T[e, 8a+j] = vT_s[e, j, 256b + 16t + a]
                    v_view = _fview(
                        vT_s[:, :], [[1, 128]], 8 * (256 * b + 16 * tt)
                    )
                    pv = psT2.tile([128, 128], F32, tag="pst2", name="pV")
                    nc.tensor.transpose(pv, v_view, ident)
                    nc.vector.tensor_copy(out=V_sb[b][:, tt, :], in_=pv)
            if debug:
                nc.sync.dma_start(out=dbg["qT"][:, :], in_=qT_s.bitcast(F32))
                nc.sync.dma_start(out=dbg["KT"][:, :, :], in_=KT_sb[0].bitcast(F32))
                nc.sync.dma_start(out=dbg["V"][:, :, :], in_=V_sb[0].bitcast(F32))
        kv_es.close()  # vT/k-stream dead once KT/V tiles exist

        # Wo preload: region reuses the kv pool space (freed at P3 end), so
        # this 4MB DMA overlaps the whole attention phase. Lives in qkv_es,
        # whose close moves to after the Wo phase to keep LIFO order.
        wopool = qkv_es.enter_context(tc.tile_pool(name="wotile", bufs=1))
        wo_s = wopool.tile([128, 8, D], F32R, name="wo_s")
        nc.sync.dma_start(
            out=wo_s,
            in_=t["wo_d"][:, :].rearrange("(j p) n -> p j n", p=128).bitcast(F32R),
        )
        g1b = bcast_load(wopool, t["g1_d"], "g1b")
        be1b = bcast_load(wopool, t["be1_d"], "be1b")

        def layer_norm(dest, pre, gb, bb, pool):
            """dest = LN(pre) * gb + bb ; pre is [128, 1024] SBUF."""
            st = pool.tile([128, 2, 6], F32, tag="bnst", name="st")
            nc.vector.bn_stats(out=st[:, 0, :], in_=pre[:, 0:512])
            nc.vector.bn_stats(out=st[:, 1, :], in_=pre[:, 512:1024])
            mv = pool.tile([128, 2], F32, tag="bnmv", name="mv")
            nc.vector.bn_aggr(out=mv, in_=st)
            rstd = pool.tile([128, 1], F32, tag="rstd", name="rstd")
            nc.scalar.activation(
                out=rstd,
                in_=mv[:, 1:2],
                func=mybir.ActivationFunctionType.Sqrt,
                bias=eps_t,
            )
            nc.vector.reciprocal(out=rstd, in_=rstd)
            xn = pool.tile([128, D], F32, tag="xn", name="xn")
            nc.vector.tensor_scalar(
                out=xn,
                in0=pre,
                scalar1=mv[:, 0:1],
                scalar2=rstd,
                op0=mybir.AluOpType.subtract,
                op1=mybir.AluOpType.mult,
            )
            tmp = pool.tile([128, D], F32, tag="pre", name="tmp")
            nc.vector.tensor_mul(out=tmp, in0=xn, in1=gb)
            nc.vector.tensor_add(out=dest, in0=tmp, in1=bb)

        # ---- phases 4+5 fused: attention, then per-block Wo + LN1 -------
        # Wo shares the attention pool scope so block 0's Wo matmuls overlap
        # block 1's attention (PSUM: S 2 + C 2 + D 2 + Wo 2 = 8 banks).
        with (
            tc.tile_pool(name="psS", bufs=2, space="PSUM") as psS,
            tc.tile_pool(name="psC", bufs=1, space="PSUM") as psC,
            tc.tile_pool(name="psD", bufs=1, space="PSUM") as psD,
            tc.tile_pool(name="psWo", bufs=2, space="PSUM") as psWo,
            tc.tile_pool(name="epool", bufs=3) as epool,
            tc.tile_pool(name="inv", bufs=2) as invp,
            tc.tile_pool(name="lnp", bufs=2) as lnp,
        ):
            for b in range(NBLK if "attn" in phases else 0):
                for lh in range(2):
                    pC = psC.tile([128, 1024], F32, tag="pc", name="pC")
                    pD = psD.tile([128, 1024], F32, tag="pd", name="pD")
                    for tt in range(16):
                        e_t = epool.tile([128, 1024], F32R, tag="e", name="e_t")
                        for q in range(2):
                            lq = slice(q * 512, (q + 1) * 512)
                            pS = psS.tile([128, 512], F32, tag="ps", name="pS")
                            # l = 1024*lh + 512*q + 8r + j ; r0 = 128*lh + 64*q
                            off = 8 * (256 * b + 128 * lh + 64 * q)
                            qt_view = qT_s[:, off : off + 512]
                            nc.tensor.matmul(
                                pS,
                                _r(KT_sb[b][:, tt, :]),
                                _r(qt_view),
                                start=True,
                                stop=False,
                            )
                            nc.tensor.matmul(
                                pS,
                                _r(rkR_s[:, tt * 128 : (tt + 1) * 128]),
                                _r(rqT_s[:, 1024 * lh + 512 * q :][:, :512]),
                                start=False,
                                stop=True,
                            )
                            nc.scalar.activation(
                                out=e_t[:, lq],
                                in_=pS,
                                func=mybir.ActivationFunctionType.Exp,
                                scale=EXP_SCALE,
                            )
                        if debug and b == 0 and lh == 0 and tt == 0:
                            nc.sync.dma_start(out=dbg["E"][:, :], in_=e_t.bitcast(F32))
                        for q in range(2):
                            lq = slice(q * 512, (q + 1) * 512)
                            nc.tensor.matmul(
                                pC[:, lq],
                                _r(V_sb[b][:, tt, :]),
                                _r(e_t[:, lq]),
                                start=(tt == 0),
                                stop=(tt == 15),
                            )
                            nc.tensor.matmul(
                                pD[:, lq],
                                _r(ones_sb),
                                _r(e_t[:, lq]),
                                start=(tt == 0),
                                stop=(tt == 15),
                            )
                    inv_t = invp.tile([128, 1024], F32, tag="inv", name="inv_t")
                    nc.vector.reciprocal(out=inv_t, in_=pD)
                    nc.vector.tensor_mul(
                        out=CT_s[b][:, 1024 * lh : 1024 * (lh + 1)],
                        in0=pC,
                        in1=inv_t,
                    )
                    # Wo + residual + LN1 for this 128-row chunk: its Wo
                    # matmuls read only the lh half of CT, so they overlap
                    # the other lh / next block instead of serializing after.
                    rc2 = lh
                    if "wo" not in phases:
                        continue
                    a = 2 * b + rc2  # core row-chunk index
                    xrow = lnp.tile([128, D], F32, tag="xrow2", name="xrow2")
                    nc.sync.dma_start(
                        out=xrow, in_=x_d[a * 128 : (a + 1) * 128, :]
                    )
                    pre = lnp.tile([128, D], F32, tag="pre", name="pre")
                    for nchunk in range(2):
                        ph = psWo.tile([128, 512], F32, tag="pswo", name="ph")
                        for j in range(8):
                            ctx_view = _fview(
                                CT_s[b][:, :], [[8, 128]], 1024 * rc2 + j
                            )
                            nc.tensor.matmul(
                                ph,
                                _r(ctx_view),
                                _r(wo_s[:, j, nchunk * 512 : (nchunk + 1) * 512]),
                                start=(j == 0),
                                stop=(j == 7),
                            )
                        nc.vector.tensor_add(
                            out=pre[:, nchunk * 512 : (nchunk + 1) * 512],
                            in0=ph,
                            in1=xrow[:, nchunk * 512 : (nchunk + 1) * 512],
                        )
                    layer_norm(h1_s[:, a, :], pre, g1b, be1b, lnp)
            if debug:
                nc.sync.dma_start(out=dbg["CT"][:, :], in_=CT_s[0].bitcast(F32))
                nc.sync.dma_start(out=dbg["h1"][:, :, :], in_=h1_s)
        qkv_es.close()  # qT/rel/KT/V/Wo dead after Wo+LN1
        ct_es.close()  # CT dead after Wo

        # ---- FFN prelude: W2/bias prefetch + h1+b2 precompute -----------
        # Opens right after the attention pools close, so the 8MB of bf16 W2
        # streams during the h1-transpose + FFN1 phases.
        ffn_es = ctx.enter_context(contextlib.ExitStack())
        w2keep = ffn_es.enter_context(tc.tile_pool(name="w2keep", bufs=1))
        rkeep = ffn_es.enter_context(tc.tile_pool(name="rkeep", bufs=32))
        lnconst = ffn_es.enter_context(tc.tile_pool(name="lnconst", bufs=1))
        ln2p = ffn_es.enter_context(tc.tile_pool(name="ln2p", bufs=2))
        outp = ffn_es.enter_context(tc.tile_pool(name="outp", bufs=2))
        pacca_pool = ffn_es.enter_context(
            tc.tile_pool(name="psFa", bufs=1, space="PSUM")
        )

        # small bias broadcasts first so they aren't queued behind weights
        g2b = bcast_load(lnconst, t["g2_d"], "g2b")
        be2b = bcast_load(lnconst, t["be2_d"], "be2b")
        b2b = bcast_load(lnconst, t["b2_d"], "b2b")

        def load_w2_group(fg):
            w2g = w2keep.tile([128, 4, D], BF16, tag=f"w2_{fg}", name=f"w2g{fg}")
            nc.sync.dma_start(
                out=w2g,
                in_=t["w2_d"][fg * 512 : (fg + 1) * 512, :].rearrange(
                    "(g p) c -> p g c", p=128
                ),
            )
            return w2g

        w1pool = ffn_es.enter_context(tc.tile_pool(name="w1tile", bufs=16))

        def load_w1_group(fq, w1rows):
            for ci in range(8):
                wt = w1pool.tile([128, 1024], BF16, tag="w1", name="w1t")
                nc.sync.dma_start(
                    out=wt,
                    in_=t["w1_d"][
                        ci * 128 : (ci + 1) * 128, fq * 1024 : (fq + 1) * 1024
                    ],
                )
                w1rows[ci] = wt

        w2_tiles = [None] * 8
        w2_tiles[0] = load_w2_group(0)
        w1rows = [None] * 8
        load_w1_group(0, w1rows)
        w2_tiles[1] = load_w2_group(1)

        # ---- phase 6: h1T (bf16); chunks 0,1 transpose while LN1 of the
        # last block is still finishing on DVE ----------------------------
        with tc.tile_pool(name="psT3", bufs=2, space="PSUM") as psT3:
            for a in range(4):
                for ct_i in range(8):
                    p = psT3.tile([128, 128], F32, tag="pst3", name="pH")
                    nc.tensor.transpose(
                        p, h1_s[:, a, ct_i * 128 : (ct_i + 1) * 128], ident
                    )
                    nc.vector.tensor_copy(
                        out=h1T_s[:, ct_i, a * 128 : (a + 1) * 128], in_=p
                    )

        # h1+b2 precompute for LN2 (emitted after the h1T copies so the DVE
        # stream doesn't stall FFN1's start on the b2 broadcast)
        h1b2_s = lnconst.tile([128, 4, D], F32, name="h1b2_s")
        for a in range(4):
            nc.vector.tensor_add(out=h1b2_s[:, a, :], in0=h1_s[:, a, :], in1=b2b)

        # ---- phase 7: FFN1+relu with FFN2 rows 0-255 interleaved --------
        pacc_a = [
            pacca_pool.tile([128, 512], F32, tag=f"psfa_{i}", name=f"psfa_{i}")
            for i in range(4)
        ]
        relu_tiles = [None] * 32

        def ffn2_rows(f, pacc_pair01, a_base):
            fg, fl = f // 4, f % 4
            rl_t = relu_tiles[f]
            for cchunk in range(2):
                for ai in range(2):
                    nc.tensor.matmul(
                        pacc_pair01[ai * 2 + cchunk],
                        rl_t[:, (a_base + ai) * 128 : (a_base + ai + 1) * 128],
                        w2_tiles[fg][:, fl, cchunk * 512 : (cchunk + 1) * 512],
                        start=(f == 0),
                        stop=(f == 31),
                    )

        def ln2_out(a, pacc_pair):
            pre2 = ln2p.tile([128, D], F32, tag="pre", name="pre2")
            for cchunk in range(2):
                cs = slice(cchunk * 512, (cchunk + 1) * 512)
                nc.vector.tensor_add(
                    out=pre2[:, cs],
                    in0=pacc_pair[cchunk],
                    in1=h1b2_s[:, a, cs],
                )
            o_t = outp.tile([128, D], F32, tag="o", name="o_t")
            layer_norm(o_t, pre2, g2b, be2b, ln2p)
            nc.sync.dma_start(out=out_d[a * 128 : (a + 1) * 128, :], in_=o_t)

        with tc.tile_pool(name="psF1", bufs=4, space="PSUM") as psF1:
            w1next = [None] * 8
            for f in range(32 if "ffn1" in phases else 0):
                fq, fl = f // 8, f % 8
                if fl == 0 and fq > 0:
                    w1rows, w1next = w1next, w1rows
                # prefetch: next W1 group two tiles in, next W2 group mid-way
                if fl == 2 and fq < 3:
                    load_w1_group(fq + 1, w1next)
                if f % 4 == 2 and f // 4 + 2 <= 7:
                    w2_tiles[f // 4 + 2] = load_w2_group(f // 4 + 2)
                pm = psF1.tile([128, ROWS], F32, tag="psf1", name="pF")
                for ci in range(8):
                    nc.tensor.matmul(
                        pm,
                        w1rows[ci][:, fl * 128 : (fl + 1) * 128],
                        h1T_s[:, ci, :],
                        start=(ci == 0),
                        stop=(ci == 7),
                    )
                rt = rkeep.tile([128, ROWS], BF16, tag="rkeep", name="rk")
                relu_tiles[f] = rt
                nc.scalar.activation(
                    out=rt,
                    in_=pm,
                    func=mybir.ActivationFunctionType.Relu,
                    bias=b1t[:, f : f + 1],
                )
                if f > 0 and "ffn2" in phases:
                    ffn2_rows(f - 1, pacc_a, 0)
            if "ffn2" in phases:
                ffn2_rows(31, pacc_a, 0)

        # ---- phase 8: FFN2 rows 256-511 (PE) overlapped with LN2 a=0,1 --
        with tc.tile_pool(name="psFb", bufs=1, space="PSUM") as paccb_pool:
            pacc_b = [
                paccb_pool.tile([128, 512], F32, tag=f"psfb_{i}", name=f"psfb_{i}")
                for i in range(4)
            ]
            for a in range(2 if "ffn2" in phases else 0):
                ln2_out(a, pacc_a[2 * a : 2 * a + 2])
            for f in range(32 if "ffn2" in phases else 0):
                ffn2_rows(f, pacc_b, 2)
            for a in range(2, 4) if "ffn2" in phases else ():
                ln2_out(a, pacc_b[2 * (a - 2) : 2 * (a - 2) + 2])


def _get_nc(debug=False):
    key = ("dbg" if debug else "main")
    if key not in _cache:
        _cache[key] = build_nc(debug)
    return _cache[key]


def kernel(**inputs):
    import ml_dtypes

    h = np.ascontiguousarray(np.asarray(inputs["h"], dtype=np.float32))
    rh = np.ascontiguousarray(np.asarray(inputs["rh"], dtype=np.float32))
    weights = {
        k: np.ascontiguousarray(np.asarray(inputs[k], dtype=np.float32))
        for k in (
            "Wq", "Wk", "Wv", "Wo", "Wrk", "Wrq",
            "b1", "b2", "g1", "be1", "g2", "be2",
        )
    }
    for k in ("W1", "W2"):
        weights[k] = np.ascontiguousarray(
            np.asarray(inputs[k]).astype(ml_dtypes.bfloat16)
        )
    in_maps = []
    for c in range(8):
        b, r0 = c // 4, 512 * (c % 4)
        m = {"x": h[b, r0 : r0 + 512, :], "rh": rh[b]}
        m.update(weights)
        in_maps.append(m)

    nc = _get_nc()
    res = run_bass_kernel_spmd(nc, in_maps, core_ids=list(range(8)))
    out = np.empty((B, L, D), dtype=np.float32)
    for c in range(8):
        b, r0 = c // 4, 512 * (c % 4)
        out[b, r0 : r0 + 512, :] = res.results[c]["out"]
    return out



# revision 18
# speedup vs baseline: 1.1014x; 1.0693x over previous
"""Trainium2 Bass kernel for nn_GTLayer_84722524880938.

The reference uses .reshape (not transpose) for the attention head split,
which makes attention block-diagonal over 256-row blocks of the sequence:
output rows [256n, 256n+256) depend only on input rows [256n, 256n+256)
(plus the full-length relative-position bias, which is rank-4). The layer
therefore shards perfectly across 8 cores: core c takes 512 contiguous
rows (2 blocks) of batch c//4 and needs no collectives.

Per 256-row block (X = h[b, 256n:256n+256, :]):
  q = X@Wq; k = X@Wk; v = X@Wv            [256, 1024]
  Q = q.reshape(2048, 128); KT = k.reshape(128, 2048); V = v.reshape(2048, 128)
  S = Q@KT/sqrt(128) + (rh[b]@Wrq) @ (rh[b]@Wrk).reshape(4, 2048) / 2
  P = softmax(S, -1);  C = P@V            [2048, 128]
  h_sa = C.reshape(256, 1024) @ Wo
  h1 = LN(h_sa + X);  hf = relu(h1@W1 + b1)@W2 + b2;  out = LN(h1 + hf)

All matmuls run as float32r (full fp32 data, full-rate PE mode). Scores
are exponentiated without max-subtraction (|S| < ~14, far from fp32 exp
overflow). The softmax denominator comes from an extra ones-weight matmul
accumulated alongside P@V.
"""

import sys

sys.path.insert(0, "/opt/trn_rl_repo")

import math

import numpy as np

import concourse.bass as bass
import concourse.mybir as mybir
import concourse.tile as tile
from concourse.bass_utils import run_bass_kernel_spmd
from concourse.masks import make_identity

F32 = mybir.dt.float32
F32R = mybir.dt.float32r
BF16 = mybir.dt.bfloat16

D, FFN, NH, HD, RL = 1024, 4096, 8, 128, 4
B, L = 2, 2048
ROWS = 512  # rows per core
NBLK = 2  # 256-row attention blocks per core
EPS = 1e-5
EXP_SCALE = 1.0 / math.sqrt(HD)  # applied by ACT on scores
RK_SCALE = math.sqrt(HD) / 2.0  # folded into r_k so bias lands as bias/2

MAX_WAITS = 1  # this walrus build allows one semaphore wait per instruction

_cache = {}


def _fix_waits(nc):
    """Split >MAX_WAITS sync waits onto injected same-engine NoOps.

    Engines execute their stream in order, so hoisting excess waits onto
    NoOps placed immediately before the instruction preserves semantics.
    """
    ctr = 0
    for f in nc.m.functions:
        for blk in f.blocks:
            out = []
            changed = False
            for ins in blk.instructions:
                si = ins.sync_info
                waits = list(si.on_wait) if si is not None else []
                if len(waits) > MAX_WAITS:
                    changed = True
                    while len(waits) > MAX_WAITS:
                        chunk, waits = waits[:MAX_WAITS], waits[MAX_WAITS:]
                        ctr += 1
                        nop = mybir.InstNoOp(
                            name=f"waitfix-nop-{ctr}",
                            ins=[],
                            outs=[],
                            sync_info=mybir.SyncInfo(on_wait=chunk, on_update=[]),
                        )
                        nop.engine = ins.engine
                        out.append(nop)
                    ins.sync_info = mybir.SyncInfo(
                        on_wait=waits, on_update=list(si.on_update)
                    )
                out.append(ins)
            if changed:
                blk.instructions = out
    return nc


def _r(ap):
    return ap.bitcast(F32R)


def _fview(base, free_dims, extra_off=0):
    """Rebuild an AP keeping the partition dim, with custom free dims/offset."""
    return bass.AP(
        tensor=base.tensor,
        offset=base.offset + extra_off,
        ap=[list(base.ap[0])] + [list(d) for d in free_dims],
    )


def build_nc(debug=False, repeat=1, phases=None):
    nc = bass.Bass(target_bir_lowering=False)

    x_d = nc.dram_tensor("x", [ROWS, D], F32, kind="ExternalInput")
    rh_d = nc.dram_tensor("rh", [L, RL], F32, kind="ExternalInput")
    wq_d = nc.dram_tensor("Wq", [D, D], BF16, kind="ExternalInput")
    wk_d = nc.dram_tensor("Wk", [D, D], BF16, kind="ExternalInput")
    wv_d = nc.dram_tensor("Wv", [D, D], BF16, kind="ExternalInput")
    wo_d = nc.dram_tensor("Wo", [D, D], BF16, kind="ExternalInput")
    wrk_d = nc.dram_tensor("Wrk", [RL, RL], F32, kind="ExternalInput")
    wrq_d = nc.dram_tensor("Wrq", [RL, RL], F32, kind="ExternalInput")
    w1_d = nc.dram_tensor("W1", [D, FFN], BF16, kind="ExternalInput")
    b1_d = nc.dram_tensor("b1", [FFN], F32, kind="ExternalInput")
    w2_d = nc.dram_tensor("W2", [FFN, D], BF16, kind="ExternalInput")
    b2_d = nc.dram_tensor("b2", [D], F32, kind="ExternalInput")
    g1_d = nc.dram_tensor("g1", [D], F32, kind="ExternalInput")
    be1_d = nc.dram_tensor("be1", [D], F32, kind="ExternalInput")
    g2_d = nc.dram_tensor("g2", [D], F32, kind="ExternalInput")
    be2_d = nc.dram_tensor("be2", [D], F32, kind="ExternalInput")
    out_d = nc.dram_tensor("out", [ROWS, D], F32, kind="ExternalOutput")

    dbg = {}
    if debug:
        dbg["qT"] = nc.dram_tensor("dbg_qT", [128, NH * ROWS], F32, kind="ExternalOutput")
        dbg["KT"] = nc.dram_tensor("dbg_KT", [128, 16, 128], F32, kind="ExternalOutput")
        dbg["V"] = nc.dram_tensor("dbg_V", [128, 16, 128], F32, kind="ExternalOutput")
        dbg["rkR"] = nc.dram_tensor("dbg_rkR", [RL, L], F32, kind="ExternalOutput")
        dbg["rqT"] = nc.dram_tensor("dbg_rqT", [RL, L], F32, kind="ExternalOutput")
        dbg["E"] = nc.dram_tensor("dbg_E", [128, 1024], F32, kind="ExternalOutput")
        dbg["CT"] = nc.dram_tensor("dbg_CT", [128, L], F32, kind="ExternalOutput")
        dbg["h1"] = nc.dram_tensor("dbg_h1", [128, 4, D], F32, kind="ExternalOutput")
        dbg["relu"] = nc.dram_tensor("dbg_relu", [128, ROWS], F32, kind="ExternalOutput")

    ph = phases
    with tile.TileContext(nc, pool_alloc_mode="stack") as tc:
        for _rep in range(repeat):
            _body(nc, tc, locals())

    _fix_waits(nc)
    return nc


def _body(nc, tc, t):
    phases = t["ph"] or {"qkv", "ktv", "attn", "wo", "ffn1", "ffn2"}
    debug = t["debug"]
    dbg = t["dbg"]
    x_d, rh_d, out_d = t["x_d"], t["rh_d"], t["out_d"]

    import contextlib

    ctx = contextlib.ExitStack()
    with ctx:
        # ---- pools ordered by lifetime (longest-lived first) ------------
        singles = ctx.enter_context(tc.tile_pool(name="singles", bufs=1))
        h1T_es = ctx.enter_context(contextlib.ExitStack())
        ct_es = h1T_es.enter_context(contextlib.ExitStack())
        qkv_es = ct_es.enter_context(contextlib.ExitStack())
        kv_es = qkv_es.enter_context(contextlib.ExitStack())

        ident = singles.tile([128, 128], F32)
        make_identity(nc, ident)
        ident_r = ident.bitcast(F32R)
        ident_bf = singles.tile([128, 128], BF16, name="ident_bf")
        nc.vector.tensor_copy(out=ident_bf, in_=ident)
        ones_sb = singles.tile([128, 128], BF16, name="ones_sb")
        nc.vector.memset(ones_sb, 1.0)
        eps_t = singles.tile([128, 1], F32)
        nc.vector.memset(eps_t, EPS)

        def bcast_load(pool, dram, name):
            tl = pool.tile([128, D], F32, name=name, tag=name)
            src = bass.AP(tensor=dram, offset=0, ap=[[0, 128], [1, D]])
            nc.sync.dma_start(out=tl, in_=src)
            return tl

        b1t = singles.tile([128, FFN // 128], F32)
        nc.sync.dma_start(
            out=b1t,
            in_=bass.AP(tensor=t["b1_d"], offset=0, ap=[[1, 128], [128, FFN // 128]]),
        )
        h1_s = singles.tile([128, 4, D], F32, name="h1_s")  # written after LN1

        h1T_pool = h1T_es.enter_context(tc.tile_pool(name="h1T", bufs=1))
        h1T_s = h1T_pool.tile([128, 8, ROWS], BF16, name="h1T_s")

        ct_pool = ct_es.enter_context(tc.tile_pool(name="ct", bufs=1))
        CT_s = [ct_pool.tile([128, L], BF16, name=f"CTb{b}", tag=f"CTb{b}") for b in range(NBLK)]

        qT_pool = qkv_es.enter_context(tc.tile_pool(name="qT", bufs=1))
        qT_s = qT_pool.tile([128, NH * ROWS], BF16, name="qT_s")
        rqT_s = qT_pool.tile([RL, L], BF16, name="rqT_s")
        rkR_s = qT_pool.tile([RL, L], BF16, name="rkR_s")
        ktv_pool = qkv_es.enter_context(tc.tile_pool(name="ktv", bufs=1))
        KT_sb = [ktv_pool.tile([128, 16, 128], BF16, name=f"KTb{b}", tag=f"KTb{b}") for b in range(NBLK)]
        V_sb = [ktv_pool.tile([128, 16, 128], BF16, name=f"Vb{b}", tag=f"Vb{b}") for b in range(NBLK)]

        kv_pool = kv_es.enter_context(tc.tile_pool(name="kv", bufs=1))
        vT_s = kv_pool.tile([128, NH * ROWS], BF16, name="vT_s")
        kstream = kv_es.enter_context(tc.tile_pool(name="kstream", bufs=3))

        # ---- phase 1+2: XT, rel-bias, q/k/v; then KT/V tiles ------------
        with (
            tc.tile_pool(name="xt", bufs=1) as xt_pool,
            tc.tile_pool(name="psT", bufs=2, space="PSUM") as psT,
            tc.tile_pool(name="psMM", bufs=4, space="PSUM") as psMM,
            tc.tile_pool(name="psT2", bufs=2, space="PSUM") as psT2,
            tc.tile_pool(name="wtile", bufs=9) as wpool,
            tc.tile_pool(name="cpy", bufs=3) as cpy,
        ):
            xT_s = xt_pool.tile([128, 8, ROWS], BF16, name="xT_s")
            for rc in range(4):
                xrow = cpy.tile([128, D], F32, tag="xrow", name="xrow")
                nc.sync.dma_start(out=xrow, in_=x_d[rc * 128 : (rc + 1) * 128, :])
                for ct_i in range(8):
                    p = psT.tile([128, 128], F32R, tag="pst", name="pT")
                    nc.tensor.transpose(
                        p, _r(xrow[:, ct_i * 128 : (ct_i + 1) * 128]), ident_r
                    )
                    nc.vector.tensor_copy(
                        out=xT_s[:, ct_i, rc * 128 : (rc + 1) * 128], in_=p
                    )

            # rhT [4, 2048] via 16 PE transposes of [128, 4] row tiles
            rh_sb = cpy.tile([128, 16, RL], F32, tag="rh", name="rh_sb")
            nc.sync.dma_start(
                out=rh_sb, in_=rh_d[:, :].rearrange("(a p) u -> p a u", p=128)
            )
            rhT_s = xt_pool.tile([RL, L], F32R, name="rhT_s")
            for a in range(16):
                p = psT.tile([128, 128], F32R, tag="pst", name="pT2")
                nc.tensor.transpose(p[:RL, :], _r(rh_sb[:, a, :]), ident_r)
                nc.vector.tensor_copy(
                    out=rhT_s[:, a * 128 : (a + 1) * 128], in_=p[:RL, :]
                )

            # r_qT / r_kT: [4, 2048] = Wr.T @ rh.T
            wr_sb = cpy.tile([RL, 2, RL], F32R, tag="wr", name="wr_sb")
            nc.sync.dma_start(out=wr_sb[:, 0, :], in_=t["wrq_d"][:, :].bitcast(F32R))
            nc.sync.dma_start(out=wr_sb[:, 1, :], in_=t["wrk_d"][:, :].bitcast(F32R))
            rkT_s = xt_pool.tile([RL, L], BF16, name="rkT_s")
            for half in range(4):
                sl = slice(half * 512, (half + 1) * 512)
                pq = psMM.tile([128, 512], F32, tag="qkv", name="pq")[:RL, :]
                nc.tensor.matmul(
                    pq, _r(wr_sb[:, 0, :]), _r(rhT_s[:, sl]), start=True, stop=True
                )
                nc.vector.tensor_copy(out=rqT_s[:, sl], in_=pq)
                pk = psMM.tile([128, 512], F32, tag="qkv", name="pk")[:RL, :]
                nc.tensor.matmul(
                    pk, _r(wr_sb[:, 1, :]), _r(rhT_s[:, sl]), start=True, stop=True
                )
                nc.vector.tensor_scalar_mul(out=rkT_s[:, sl], in0=pk, scalar1=RK_SCALE)

            # rkR[t, 4i+u] = rkT[u, 512t+i]  (reshape(4, 2048) of r_k)
            for tt in range(4):
                for u in range(RL):
                    nc.sync.dma_start(
                        out=_fview(rkR_s[tt : tt + 1, :], [[RL, 512]], u),
                        in_=rkT_s[u : u + 1, tt * 512 : (tt + 1) * 512],
                    )

            if debug:
                nc.sync.dma_start(out=dbg["rkR"][:, :], in_=rkR_s.bitcast(F32))
                nc.sync.dma_start(out=dbg["rqT"][:, :], in_=rqT_s.bitcast(F32))

            # q and v full [128, 8, ROWS]; weights loaded as [128, 512] half-rows
            for w_d, dest in ((t["wq_d"], qT_s), (t["wv_d"], vT_s)) if "qkv" in phases else ():
                for half in range(2):
                    wrows = [None] * 8
                    for ci in range(8):
                        wrow = wpool.tile([128, 512], BF16, tag="w", name="wrow")
                        nc.sync.dma_start(
                            out=wrow,
                            in_=w_d[
                                ci * 128 : (ci + 1) * 128,
                                half * 512 : (half + 1) * 512,
                            ],
                        )
                        wrows[ci] = wrow
                    for col in range(4):
                        co = half * 4 + col
                        pm = psMM.tile([128, ROWS], F32, tag="qkv", name="pm")
                        for ci in range(8):
                            nc.tensor.matmul(
                                pm,
                                wrows[ci][:, col * 128 : (col + 1) * 128],
                                xT_s[:, ci, :],
                                start=(ci == 0),
                                stop=(ci == 7),
                            )
                        nc.vector.tensor_copy(
                            out=_fview(dest[:, :], [[8, ROWS]], co), in_=pm
                        )

            # k per-co streaming: each co slice feeds only KT tiles t%8==co
            for half in range(2 if "qkv" in phases else 0):
                wrows = [None] * 8
                for ci in range(8):
                    wrow = wpool.tile([128, 512], BF16, tag="w", name="wkrow")
                    nc.sync.dma_start(
                        out=wrow,
                        in_=t["wk_d"][
                            ci * 128 : (ci + 1) * 128, half * 512 : (half + 1) * 512
                        ],
                    )
                    wrows[ci] = wrow
                for col in range(4):
                    co = half * 4 + col
                    pm = psMM.tile([128, ROWS], F32, tag="qkv", name="pmk")
                    for ci in range(8):
                        nc.tensor.matmul(
                            pm,
                            wrows[ci][:, col * 128 : (col + 1) * 128],
                            xT_s[:, ci, :],
                            start=(ci == 0),
                            stop=(ci == 7),
                        )
                    kco = kstream.tile([128, ROWS], BF16, tag="kco", name="kco")
                    nc.vector.tensor_copy(out=kco, in_=pm)
                    for b in range(NBLK):
                        for tt in (co, co + 8):
                            # KT_t^T[mm, d] = k[256b + 2d + (t>=8), 128co + mm]
                            kt_view = _fview(
                                kco[:, :], [[2, 128]], 256 * b + (1 if tt >= 8 else 0)
                            )
                            p = psT2.tile([128, 128], BF16, tag="pst2", name="pKT")
                            nc.tensor.transpose(p, kt_view, ident_bf)
                            nc.vector.tensor_copy(out=KT_sb[b][:, tt, :], in_=p)

            # V tiles from vT_s
            for b in range(NBLK if "ktv" in phases else 0):
                for tt in range(16):
                    # V_t^T[e, 8a+j] = vT_s[e, j, 256b + 16t + a]
                    v_view = _fview(
                        vT_s[:, :], [[1, 128]], 8 * (256 * b + 16 * tt)
                    )
                    pv = psT2.tile([128, 128], BF16, tag="pst2", name="pV")
                    nc.tensor.transpose(pv, v_view, ident_bf)
                    nc.vector.tensor_copy(out=V_sb[b][:, tt, :], in_=pv)
            if debug:
                nc.sync.dma_start(out=dbg["qT"][:, :], in_=qT_s.bitcast(F32))
                nc.sync.dma_start(out=dbg["KT"][:, :, :], in_=KT_sb[0].bitcast(F32))
                nc.sync.dma_start(out=dbg["V"][:, :, :], in_=V_sb[0].bitcast(F32))
        kv_es.close()  # vT/k-stream dead once KT/V tiles exist

        # Wo preload: region reuses the kv pool space (freed at P3 end), so
        # this 4MB DMA overlaps the whole attention phase. Lives in qkv_es,
        # whose close moves to after the Wo phase to keep LIFO order.
        wopool = qkv_es.enter_context(tc.tile_pool(name="wotile", bufs=1))
        wo_s = wopool.tile([128, 8, D], BF16, name="wo_s")
        nc.sync.dma_start(
            out=wo_s,
            in_=t["wo_d"][:, :].rearrange("(j p) n -> p j n", p=128),
        )
        g1b = bcast_load(wopool, t["g1_d"], "g1b")
        be1b = bcast_load(wopool, t["be1_d"], "be1b")

        def layer_norm(dest, pre, gb, bb, pool):
            """dest = LN(pre) * gb + bb ; pre is [128, 1024] SBUF."""
            st = pool.tile([128, 2, 6], F32, tag="bnst", name="st")
            nc.vector.bn_stats(out=st[:, 0, :], in_=pre[:, 0:512])
            nc.vector.bn_stats(out=st[:, 1, :], in_=pre[:, 512:1024])
            mv = pool.tile([128, 2], F32, tag="bnmv", name="mv")
            nc.vector.bn_aggr(out=mv, in_=st)
            rstd = pool.tile([128, 1], F32, tag="rstd", name="rstd")
            nc.scalar.activation(
                out=rstd,
                in_=mv[:, 1:2],
                func=mybir.ActivationFunctionType.Sqrt,
                bias=eps_t,
            )
            nc.vector.reciprocal(out=rstd, in_=rstd)
            xn = pool.tile([128, D], F32, tag="xn", name="xn")
            nc.vector.tensor_scalar(
                out=xn,
                in0=pre,
                scalar1=mv[:, 0:1],
                scalar2=rstd,
                op0=mybir.AluOpType.subtract,
                op1=mybir.AluOpType.mult,
            )
            tmp = pool.tile([128, D], F32, tag="pre", name="tmp")
            nc.vector.tensor_mul(out=tmp, in0=xn, in1=gb)
            nc.vector.tensor_add(out=dest, in0=tmp, in1=bb)

        # ---- phases 4+5 fused: attention, then per-block Wo + LN1 -------
        # Wo shares the attention pool scope so block 0's Wo matmuls overlap
        # block 1's attention (PSUM: S 2 + C 2 + D 2 + Wo 2 = 8 banks).
        with (
            tc.tile_pool(name="psS", bufs=2, space="PSUM") as psS,
            tc.tile_pool(name="psC", bufs=1, space="PSUM") as psC,
            tc.tile_pool(name="psD", bufs=1, space="PSUM") as psD,
            tc.tile_pool(name="psWo", bufs=2, space="PSUM") as psWo,
            tc.tile_pool(name="epool", bufs=3) as epool,
            tc.tile_pool(name="inv", bufs=2) as invp,
            tc.tile_pool(name="lnp", bufs=2) as lnp,
        ):
            for b in range(NBLK if "attn" in phases else 0):
                for lh in range(2):
                    pC = psC.tile([128, 1024], F32, tag="pc", name="pC")
                    pD = psD.tile([128, 1024], F32, tag="pd", name="pD")
                    dacc = invp.tile([128, 1024], BF16, tag="dacc", name="dacc")
                    for tt in range(16):
                        e_t = epool.tile([128, 1024], BF16, tag="e", name="e_t")
                        for q in range(2):
                            lq = slice(q * 512, (q + 1) * 512)
                            pS = psS.tile([128, 512], F32, tag="ps", name="pS")
                            # l = 1024*lh + 512*q + 8r + j ; r0 = 128*lh + 64*q
                            off = 8 * (256 * b + 128 * lh + 64 * q)
                            qt_view = qT_s[:, off : off + 512]
                            nc.tensor.matmul(
                                pS,
                                KT_sb[b][:, tt, :],
                                qt_view,
                                start=True,
                                stop=False,
                            )
                            nc.tensor.matmul(
                                pS,
                                rkR_s[:, tt * 128 : (tt + 1) * 128],
                                rqT_s[:, 1024 * lh + 512 * q :][:, :512],
                                start=False,
                                stop=True,
                            )
                            nc.scalar.activation(
                                out=e_t[:, lq],
                                in_=pS,
                                func=mybir.ActivationFunctionType.Exp,
                                scale=EXP_SCALE,
                            )
                        if debug and b == 0 and lh == 0 and tt == 0:
                            nc.sync.dma_start(out=dbg["E"][:, :], in_=e_t.bitcast(F32))
                        # softmax denominator: bf16 running sum on DVE (2x
                        # mode) replaces a second 512-cycle PE matmul per tile
                        if tt == 0:
                            nc.vector.tensor_copy(out=dacc, in_=e_t)
                        else:
                            nc.vector.tensor_add(out=dacc, in0=dacc, in1=e_t)
                        for q in range(2):
                            lq = slice(q * 512, (q + 1) * 512)
                            nc.tensor.matmul(
                                pC[:, lq],
                                V_sb[b][:, tt, :],
                                e_t[:, lq],
                                start=(tt == 0),
                                stop=(tt == 15),
                            )
                    nc.tensor.matmul(pD, ones_sb, dacc, start=True, stop=True)
                    inv_t = invp.tile([128, 1024], F32, tag="inv", name="inv_t")
                    nc.vector.reciprocal(out=inv_t, in_=pD)
                    nc.vector.tensor_mul(
                        out=CT_s[b][:, 1024 * lh : 1024 * (lh + 1)],
                        in0=pC,
                        in1=inv_t,
                    )
                    # Wo + residual + LN1 for this 128-row chunk: its Wo
                    # matmuls read only the lh half of CT, so they overlap
                    # the other lh / next block instead of serializing after.
                    rc2 = lh
                    if "wo" not in phases:
                        continue
                    a = 2 * b + rc2  # core row-chunk index
                    xrow = lnp.tile([128, D], F32, tag="xrow2", name="xrow2")
                    nc.sync.dma_start(
                        out=xrow, in_=x_d[a * 128 : (a + 1) * 128, :]
                    )
                    pre = lnp.tile([128, D], F32, tag="pre", name="pre")
                    for nchunk in range(2):
                        ph = psWo.tile([128, 512], F32, tag="pswo", name="ph")
                        for j in range(8):
                            ctx_view = _fview(
                                CT_s[b][:, :], [[8, 128]], 1024 * rc2 + j
                            )
                            nc.tensor.matmul(
                                ph,
                                ctx_view,
                                wo_s[:, j, nchunk * 512 : (nchunk + 1) * 512],
                                start=(j == 0),
                                stop=(j == 7),
                            )
                        nc.vector.tensor_add(
                            out=pre[:, nchunk * 512 : (nchunk + 1) * 512],
                            in0=ph,
                            in1=xrow[:, nchunk * 512 : (nchunk + 1) * 512],
                        )
                    layer_norm(h1_s[:, a, :], pre, g1b, be1b, lnp)
            if debug:
                nc.sync.dma_start(out=dbg["CT"][:, :], in_=CT_s[0].bitcast(F32))
                nc.sync.dma_start(out=dbg["h1"][:, :, :], in_=h1_s)
        qkv_es.close()  # qT/rel/KT/V/Wo dead after Wo+LN1
        ct_es.close()  # CT dead after Wo

        # ---- FFN prelude: W2/bias prefetch + h1+b2 precompute -----------
        # Opens right after the attention pools close, so the 8MB of bf16 W2
        # streams during the h1-transpose + FFN1 phases.
        ffn_es = ctx.enter_context(contextlib.ExitStack())
        w2keep = ffn_es.enter_context(tc.tile_pool(name="w2keep", bufs=1))
        rkeep = ffn_es.enter_context(tc.tile_pool(name="rkeep", bufs=32))
        lnconst = ffn_es.enter_context(tc.tile_pool(name="lnconst", bufs=1))
        ln2p = ffn_es.enter_context(tc.tile_pool(name="ln2p", bufs=2))
        outp = ffn_es.enter_context(tc.tile_pool(name="outp", bufs=2))
        pacca_pool = ffn_es.enter_context(
            tc.tile_pool(name="psFa", bufs=1, space="PSUM")
        )

        # small bias broadcasts first so they aren't queued behind weights
        g2b = bcast_load(lnconst, t["g2_d"], "g2b")
        be2b = bcast_load(lnconst, t["be2_d"], "be2b")
        b2b = bcast_load(lnconst, t["b2_d"], "b2b")

        def load_w2_group(fg):
            w2g = w2keep.tile([128, 4, D], BF16, tag=f"w2_{fg}", name=f"w2g{fg}")
            nc.sync.dma_start(
                out=w2g,
                in_=t["w2_d"][fg * 512 : (fg + 1) * 512, :].rearrange(
                    "(g p) c -> p g c", p=128
                ),
            )
            return w2g

        w1pool = ffn_es.enter_context(tc.tile_pool(name="w1tile", bufs=16))

        def load_w1_group(fq, w1rows):
            for ci in range(8):
                wt = w1pool.tile([128, 1024], BF16, tag="w1", name="w1t")
                nc.sync.dma_start(
                    out=wt,
                    in_=t["w1_d"][
                        ci * 128 : (ci + 1) * 128, fq * 1024 : (fq + 1) * 1024
                    ],
                )
                w1rows[ci] = wt

        w2_tiles = [None] * 8
        w2_tiles[0] = load_w2_group(0)
        w1rows = [None] * 8
        load_w1_group(0, w1rows)
        w2_tiles[1] = load_w2_group(1)

        # ---- phase 6: h1T (bf16); chunks 0,1 transpose while LN1 of the
        # last block is still finishing on DVE ----------------------------
        with tc.tile_pool(name="psT3", bufs=2, space="PSUM") as psT3:
            for a in range(4):
                for ct_i in range(8):
                    p = psT3.tile([128, 128], F32R, tag="pst3", name="pH")
                    nc.tensor.transpose(
                        p, _r(h1_s[:, a, ct_i * 128 : (ct_i + 1) * 128]), ident_r
                    )
                    nc.vector.tensor_copy(
                        out=h1T_s[:, ct_i, a * 128 : (a + 1) * 128], in_=p
                    )

        # h1+b2 precompute for LN2 (emitted after the h1T copies so the DVE
        # stream doesn't stall FFN1's start on the b2 broadcast)
        h1b2_s = lnconst.tile([128, 4, D], F32, name="h1b2_s")
        for a in range(4):
            nc.vector.tensor_add(out=h1b2_s[:, a, :], in0=h1_s[:, a, :], in1=b2b)

        # ---- phase 7: FFN1+relu with FFN2 rows 0-255 interleaved --------
        pacc_a = [
            pacca_pool.tile([128, 512], F32, tag=f"psfa_{i}", name=f"psfa_{i}")
            for i in range(4)
        ]
        relu_tiles = [None] * 32

        def ffn2_rows(f, pacc_pair01, a_base):
            fg, fl = f // 4, f % 4
            rl_t = relu_tiles[f]
            for cchunk in range(2):
                for ai in range(2):
                    nc.tensor.matmul(
                        pacc_pair01[ai * 2 + cchunk],
                        rl_t[:, (a_base + ai) * 128 : (a_base + ai + 1) * 128],
                        w2_tiles[fg][:, fl, cchunk * 512 : (cchunk + 1) * 512],
                        start=(f == 0),
                        stop=(f == 31),
                    )

        def ln2_out(a, pacc_pair):
            pre2 = ln2p.tile([128, D], F32, tag="pre", name="pre2")
            for cchunk in range(2):
                cs = slice(cchunk * 512, (cchunk + 1) * 512)
                nc.vector.tensor_add(
                    out=pre2[:, cs],
                    in0=pacc_pair[cchunk],
                    in1=h1b2_s[:, a, cs],
                )
            o_t = outp.tile([128, D], F32, tag="o", name="o_t")
            layer_norm(o_t, pre2, g2b, be2b, ln2p)
            nc.sync.dma_start(out=out_d[a * 128 : (a + 1) * 128, :], in_=o_t)

        with tc.tile_pool(name="psF1", bufs=4, space="PSUM") as psF1:
            w1next = [None] * 8
            for f in range(32 if "ffn1" in phases else 0):
                fq, fl = f // 8, f % 8
                if fl == 0 and fq > 0:
                    w1rows, w1next = w1next, w1rows
                # prefetch: next W1 group two tiles in, next W2 group mid-way
                if fl == 2 and fq < 3:
                    load_w1_group(fq + 1, w1next)
                if f % 4 == 2 and f // 4 + 2 <= 7:
                    w2_tiles[f // 4 + 2] = load_w2_group(f // 4 + 2)
                pm = psF1.tile([128, ROWS], F32, tag="psf1", name="pF")
                for ci in range(8):
                    nc.tensor.matmul(
                        pm,
                        w1rows[ci][:, fl * 128 : (fl + 1) * 128],
                        h1T_s[:, ci, :],
                        start=(ci == 0),
                        stop=(ci == 7),
                    )
                rt = rkeep.tile([128, ROWS], BF16, tag="rkeep", name="rk")
                relu_tiles[f] = rt
                nc.scalar.activation(
                    out=rt,
                    in_=pm,
                    func=mybir.ActivationFunctionType.Relu,
                    bias=b1t[:, f : f + 1],
                )
                if f > 0 and "ffn2" in phases:
                    ffn2_rows(f - 1, pacc_a, 0)
            if "ffn2" in phases:
                ffn2_rows(31, pacc_a, 0)

        # ---- phase 8: FFN2 rows 256-511 (PE) overlapped with LN2 a=0,1 --
        with tc.tile_pool(name="psFb", bufs=1, space="PSUM") as paccb_pool:
            pacc_b = [
                paccb_pool.tile([128, 512], F32, tag=f"psfb_{i}", name=f"psfb_{i}")
                for i in range(4)
            ]
            for a in range(2 if "ffn2" in phases else 0):
                ln2_out(a, pacc_a[2 * a : 2 * a + 2])
            for f in range(32 if "ffn2" in phases else 0):
                ffn2_rows(f, pacc_b, 2)
            for a in range(2, 4) if "ffn2" in phases else ():
                ln2_out(a, pacc_b[2 * (a - 2) : 2 * (a - 2) + 2])


def _get_nc(debug=False):
    key = ("dbg" if debug else "main")
    if key not in _cache:
        _cache[key] = build_nc(debug)
    return _cache[key]


def kernel(**inputs):
    import ml_dtypes

    h = np.ascontiguousarray(np.asarray(inputs["h"], dtype=np.float32))
    rh = np.ascontiguousarray(np.asarray(inputs["rh"], dtype=np.float32))
    weights = {
        k: np.ascontiguousarray(np.asarray(inputs[k], dtype=np.float32))
        for k in (
            "Wrk", "Wrq", "b1", "b2", "g1", "be1", "g2", "be2",
        )
    }
    for k in ("Wq", "Wk", "Wv", "Wo", "W1", "W2"):
        weights[k] = np.ascontiguousarray(
            np.asarray(inputs[k]).astype(ml_dtypes.bfloat16)
        )
    in_maps = []
    for c in range(8):
        b, r0 = c // 4, 512 * (c % 4)
        m = {"x": h[b, r0 : r0 + 512, :], "rh": rh[b]}
        m.update(weights)
        in_maps.append(m)

    nc = _get_nc()
    res = run_bass_kernel_spmd(nc, in_maps, core_ids=list(range(8)))
    out = np.empty((B, L, D), dtype=np.float32)
    for c in range(8):
        b, r0 = c // 4, 512 * (c % 4)
        out[b, r0 : r0 + 512, :] = res.results[c]["out"]
    return out

